# revision 18
# baseline (speedup 1.0000x reference)
"""Multi-head attention (B=2, N=2048, C=1024, H=16, D=64) on 8 TRN2 NeuronCores.

Sharding: 2 heads per core (tensor parallel over num_heads), both batch
elements on every core.  Each core computes q/k/v for its 2 heads, full
attention for those heads, and a partial output projection (row-parallel
over w_proj); the host sums the 8 f16 partial outputs and adds the bias.

Device-side dataflow per core:
  q/k:   f16 matmuls over 8 c-tiles into [128,512] PSUM blocks (2 heads x
         64 d on partitions).  q is evacuated to fp8e4m3 duplicated into
         both DoubleRow slots; k is evacuated as a compensated fp8 (hi,
         k-hi_fp8(k)) slot pair, so the score matmuls run in fp8 DoubleRow
         mode (0.5 PE cycles/row, half the f16 cost) with k exact and only
         q carrying ~3.6%-rms quantization noise.  Measured end-to-end
         max-rel-error 1.52e-2 against the f32 reference (budget 2e-2);
         with SCORES_FP8=False everything is f16 at 3.7e-4 error.
  v:     computed directly in [m, d] orientation (x tile as the stationary
         operand) one m-tile at a time, packed into vo tiles laid out
         [V_h0 | ones | V_h1] so each head's AV stationary operand is a
         [128,128] f16 slab whose ones columns produce the softmax
         denominator rows inside the same AV matmul (no PE transposes).
  attn:  one software-pipelined loop over all 128 (batch, head, n-chunk,
         m-pair) steps: two DoubleRow score matmuls (K=64) into a
         [128,2,512] PSUM tile, one ACT exp (scale folded in, f16 out;
         logits are O(3) so no max subtraction is needed), with the AV
         accumulation deferred up to 28 steps behind so PE never waits on
         ACT - across all chunk/head/batch boundaries.
  norm:  DVE reciprocal + multiply into oc (f16).
  proj:  y_partial[n,:] = oc.T @ w_proj in f16, f32 PSUM evacuated to f16,
         DMA'd out per n-tile.

Scheduling: only the first n-chunk's q/k chains of batch 0 run before the
attention step loop.  Everything else (remaining batch-0 q/k chains, both
batches' v units, batch-1 q/k, both projections) is chopped into
~0.4-0.9us PE work units and injected into the attention steps at an
adaptive rate, so the ACT engine's ~134us of exp runs underneath PE's
~145us of matmul work with minimal warmup and drain.  x DMAs are issued
n-half-major (and the qkv weight DMA is split q/k-first) so the first
chains start ~4us into the run; the deep AV lag lets PE batch-0 work run
far ahead of ACT while batch-1's AV stream drains the backlog.
GPSIMD cannot touch PSUM on this hardware, so all PSUM evacuation is on
DVE (plus ACT for the tail n-tiles, where exp is already finished).
"""

import sys

sys.path.insert(0, "/opt/trn_rl_repo")

import numpy as np

import concourse.mybir as mybir
import concourse.tile as tile
from concourse import bacc
from concourse.bass_utils import run_bass_kernel_spmd

F32 = mybir.dt.float32
F16 = mybir.dt.float16
F8 = mybir.dt.float8e4
AF = mybir.ActivationFunctionType
ALU = mybir.AluOpType
DR = mybir.MatmulPerfMode.DoubleRow

# fp8 DoubleRow score matmuls with averaged-pair quantization: BOTH q and k
# are stored as fp8 slot pairs (a, b) with a = fp8(x), b = fp8(2x - a), so
# k_a*q_a + k_b*q_b = 2*k*q + (k*eb' + q*ea') cancels the first-order
# rounding error of both operands (the exp scale absorbs the 2x).  Measured
# end-to-end max-rel-error 7.8e-3 (budget 2e-2) at the same 0.5 cyc/row PE
# cost as the v1 one-sided scheme (which measured 1.52e-2).
SCORES_FP8 = True

# Number of m-pair groups per chunk (of NG=8) whose AV accumulation runs in
# fp8 DoubleRow mode: exp writes those et tiles in fp8e4m3 and the two AV
# matmuls become two DR matmuls with v split hi/lo across the slot pair
# (v exact, et carrying ~2-3% fp8 noise on n/8 of the attention mass).
# Each increment cuts 16.4k PE cycles (~6.8us) at ~+0.5e-2*sqrt(n/8) error.
N_FP8_PAIRS = 0

B = 2
N = 2048
C = 1024
H = 16
D = 64
NCORES = 8
HPC = H // NCORES          # heads per core = 2
CT = C // 128              # c tiles = 8
NT = N // 128              # m tiles = 16
NP = NT // 2               # m pairs = 8
NCH = N // 512             # 512-wide n chunks = 4
SCALE = float(D) ** -0.5


def _build():
    nc = bacc.Bacc("TRN2")
    xT = nc.dram_tensor("xT", [B, CT, 128, N], F16, kind="ExternalInput")
    wqkv = nc.dram_tensor("wqkv", [CT, 128, 384], F16, kind="ExternalInput")
    wpT = nc.dram_tensor("wpT", [128, C], F16, kind="ExternalInput")
    y = nc.dram_tensor("y", [B, N, C], F16, kind="ExternalOutput")

    with tile.TileContext(nc) as tc:
        with tc.tile_pool(name="consts", bufs=1) as consts, \
             tc.tile_pool(name="xt", bufs=16) as xt_pool, \
             tc.tile_pool(name="qk", bufs=4) as qk_pool, \
             tc.tile_pool(name="kf", bufs=4) as kf_pool, \
             tc.tile_pool(name="vo", bufs=2) as vo_pool, \
             tc.tile_pool(name="et", bufs=32) as et_pool, \
             tc.tile_pool(name="oc", bufs=2) as oc_pool, \
             tc.tile_pool(name="rec", bufs=4) as rec_pool, \
             tc.tile_pool(name="yo", bufs=6) as yo_pool, \
             tc.tile_pool(name="pq", bufs=2, space="PSUM") as pq, \
             tc.tile_pool(name="ps", bufs=2, space="PSUM") as ps_pool, \
             tc.tile_pool(name="pav", bufs=2, space="PSUM") as pav:

            w_sb = consts.tile([128, CT, 384], F16)
            wp_sb = consts.tile([128, C], F16)

            xt = {}
            q16 = {}
            k16 = {}
            vo = {}
            vo8h = {}
            vo8l = {}
            oc_sb = {}

            def load_w(part):
                # qk columns first (phase-0 critical path, split so the
                # first half-chain only waits for c-tiles 0-3); v later
                if part == 0:
                    for ts_ in ((0, 4), (4, 8)):
                        nc.sync.dma_start(
                            out=w_sb[:, ts_[0]:ts_[1], 0:256],
                            in_=wqkv[ts_[0]:ts_[1], :, 0:256]
                            .rearrange("t p o -> p t o"))
                else:
                    nc.sync.dma_start(
                        out=w_sb[:, :, 256:384],
                        in_=wqkv[:, :, 256:384].rearrange("t p o -> p t o"))

            def load_x(b, pieces):
                # finer pieces land sooner: the first q/k chains only need
                # the first n-columns of every c-tile.
                for ct in range(CT):
                    if (b, ct) not in xt:
                        xt[b, ct] = xt_pool.tile([128, N], F16, tag="xt",
                                                 name=f"xt_{b}_{ct}")
                for lo, hi in pieces:
                    for ct in range(CT):
                        nc.sync.dma_start(out=xt[b, ct][:, lo:hi],
                                          in_=xT[b, ct][:, lo:hi])

            def emit_qk_units(b):
                """16 PE units: half-chains of 4 c-tiles for the q and k
                blocks, n-chunk-major so early chains need only early DMAs."""
                if SCORES_FP8:
                    # slot-dim layouts: averaged fp8 pairs (a, b) for both
                    q16[b] = qk_pool.tile([128, 2, N], F8, tag="qk",
                                          name=f"q16_{b}")
                    k16[b] = qk_pool.tile([128, 2, N], F8, tag="qk",
                                          name=f"k16_{b}")
                else:
                    q16[b] = qk_pool.tile([128, N], F16, tag="qk",
                                          name=f"q16_{b}")
                    k16[b] = qk_pool.tile([128, N], F16, tag="qk",
                                          name=f"k16_{b}")

                # q0/k0 first (run before the step loop); then all k chunks
                # ahead of all q chunks: the chunk-0 score m-sweep reads
                # k(nch2) at step 4 and k(nch3) at step 6, while q(nch1) is
                # first read at step 8 - emission must precede those reads
                order = [(0, 0), (1, 0), (1, 1), (1, 2), (1, 3),
                         (0, 1), (0, 2), (0, 3)]
                if True:  # keep indentation stable
                    for blk, nch in order:
                        sl = slice(nch * 512, (nch + 1) * 512)
                        psq = pq.tile([128, 512], F32, tag="pq",
                                      name=f"psq_{b}_{blk}_{nch}")

                        def half(ct0, psq=psq, blk=blk, sl=sl, b=b):
                            for ct in range(ct0, ct0 + 4):
                                nc.tensor.matmul(
                                    psq[:, :],
                                    w_sb[:, ct, blk * 128:(blk + 1) * 128],
                                    xt[b, ct][:, sl],
                                    start=(ct == 0), stop=(ct == CT - 1),
                                )
                            if ct0 != 4:
                                return
                            dst = q16[b] if blk == 0 else k16[b]
                            if not SCORES_FP8:
                                nc.vector.tensor_copy(dst[:, sl], psq[:, :])
                            else:
                                # averaged fp8 pair: a = fp8(x) and
                                # b = fp8(2x - a).  One DVE psum read
                                # (kf = 2x, exact in f16) frees the chain
                                # PSUM slot; the idle Pool engine builds
                                # a = fp8(0.5*kf) and b = fp8(kf - a)
                                # (GPSIMD cannot read PSUM).
                                kf = kf_pool.tile([128, 512], F16, tag="kf",
                                                  name=f"kf_{b}_{blk}_{sl.start}")
                                nc.vector.tensor_scalar(
                                    out=kf[:, :], in0=psq[:, :],
                                    scalar1=2.0, scalar2=None, op0=ALU.mult)
                                nc.gpsimd.tensor_scalar(
                                    out=dst[:, 0, sl], in0=kf[:, :],
                                    scalar1=0.5, scalar2=None, op0=ALU.mult)
                                nc.gpsimd.tensor_tensor(
                                    out=dst[:, 1, sl], in0=kf[:, :],
                                    in1=dst[:, 0, sl], op=ALU.subtract)

                        yield lambda h=half: h(0)
                        yield lambda h=half: h(4)

            def emit_v_units(b):
                """17 PE units: vo init, then one unit per m-tile computing
                v[m,d] directly (x as stationary operand, 8 accumulating
                128-free matmuls) and packing it into the vo layout.  The
                first N_FP8_PAIRS m-pairs additionally get a DoubleRow
                hi/lo fp8 copy (v exact across the two slot matmuls)."""
                vo[b] = vo_pool.tile([128, NT, 192], F16, tag="vo", name=f"vo_{b}")
                if N_FP8_PAIRS:
                    vo8h[b] = vo_pool.tile([128, N_FP8_PAIRS, 2, 192], F8,
                                           tag="vo8", name=f"vo8h_{b}")
                    vo8l[b] = vo_pool.tile([128, N_FP8_PAIRS, 2, 192], F8,
                                           tag="vo8", name=f"vo8l_{b}")

                def vo_init(b=b):
                    nc.gpsimd.memset(vo[b][:, :, 64:128], 1.0)
                    if N_FP8_PAIRS:
                        # hi carries the denominator ones; lo must not
                        # double-count them
                        nc.gpsimd.memset(vo8h[b][:, :, :, 64:128], 1.0)
                        nc.gpsimd.memset(vo8l[b][:, :, :, 64:128], 0.0)

                yield vo_init

                for mt in range(NT):
                    def vunit(mt=mt, b=b):
                        pv = pq.tile([128, 128], F32, tag="pq",
                                     name=f"pv_{b}_{mt}")
                        msl = slice(mt * 128, (mt + 1) * 128)
                        for ct in range(CT):
                            nc.tensor.matmul(
                                pv[:, :],
                                xt[b, ct][:, msl],
                                w_sb[:, ct, 256:384],
                                start=(ct == 0), stop=(ct == CT - 1),
                            )
                        nc.vector.tensor_copy(vo[b][:, mt, 0:64], pv[:, 0:64])
                        nc.vector.tensor_copy(vo[b][:, mt, 128:192], pv[:, 64:128])
                        if mt < 2 * N_FP8_PAIRS:
                            gp, st = divmod(mt, 2)
                            for c0, c1 in ((0, 64), (128, 192)):
                                nc.gpsimd.tensor_copy(
                                    vo8h[b][:, gp, st, c0:c1],
                                    vo[b][:, mt, c0:c1])
                                nc.gpsimd.tensor_tensor(
                                    out=vo8l[b][:, gp, st, c0:c1],
                                    in0=vo[b][:, mt, c0:c1],
                                    in1=vo8h[b][:, gp, st, c0:c1],
                                    op=ALU.subtract)

                    yield vunit

            def emit_proj_units(b, q):
                """4 PE units: one per n-tile (2 matmuls + evac + DMA out)."""
                for nt in range(q * NT // NCH, (q + 1) * NT // NCH):
                    def unit(nt=nt, b=b, q=q):
                        ysb = yo_pool.tile([128, 1024], F16, tag="yo",
                                           name=f"ysb_{b}_{nt}")
                        for och in range(2):
                            pp = pq.tile([128, 512], F32, tag="pq",
                                         name=f"pp_{b}_{nt}_{och}")
                            nc.tensor.matmul(
                                pp[:, :],
                                oc_sb[b][:, nt * 128:(nt + 1) * 128],
                                wp_sb[:, och * 512:(och + 1) * 512],
                                start=True, stop=True,
                            )
                            cp = (nc.scalar.copy
                                  if (b == 1 and q == 3 and och == nt % 2)
                                  else nc.vector.tensor_copy)
                            cp(ysb[:, och * 512:(och + 1) * 512], pp[:, :])
                        nc.sync.dma_start(
                            out=y[b, nt * 128:(nt + 1) * 128, :],
                            in_=ysb[:, :],
                        )

                    yield unit

            # ---- schedule ----
            load_w(0)
            load_x(0, [(0, 512), (512, 1024)])
            load_w(1)
            load_x(0, [(1024, 1536), (1536, 2048)])
            load_x(1, [(0, 1024), (1024, 2048)])
            nc.sync.dma_start(out=wp_sb, in_=wpT[:, :])

            qk0 = list(emit_qk_units(0))
            for u in qk0[:4]:
                u()

            uq = []          # pending PE work units
            uq.extend(("qk0", u) for u in qk0[4:])
            av_tile = {}

            NG = NP          # m-pair groups per chunk

            def attn_scores(b, hl, q, g):
                """Emit scores for one m-pair + exp; return deferred AV."""
                hs = hl * 64
                qof = q * 512
                if hl == 0 and q == 0 and g == 0:
                    oc_sb[b] = oc_pool.tile([128, N], F16, tag="oc",
                                            name=f"oc_{b}")
                if g == 0:
                    av_tile[b, hl, q] = pav.tile([128, 512], F32, tag="pav",
                                                 name=f"av_{b}_{hl}_{q}")
                av = av_tile[b, hl, q]
                s = ps_pool.tile([128, 2, 512], F32, tag="ps",
                                 name=f"s_{b}_{hl}_{q}_{g}")
                for i in range(2):
                    m_ = 2 * g + i
                    if SCORES_FP8:
                        nc.tensor.matmul(
                            s[:, i, :],
                            k16[b][hs:hs + 64, :, m_ * 128:(m_ + 1) * 128],
                            q16[b][hs:hs + 64, :, qof:qof + 512],
                            start=True, stop=True, perf_mode=DR,
                        )
                    else:
                        nc.tensor.matmul(
                            s[:, i, :],
                            k16[b][hs:hs + 64, m_ * 128:(m_ + 1) * 128],
                            q16[b][hs:hs + 64, qof:qof + 512],
                            start=True, stop=True,
                        )
                et = et_pool.tile([128, 2, 512],
                                  F8 if g < N_FP8_PAIRS else F16, tag="et",
                                  name=f"et_{b}_{hl}_{q}_{g}")
                nc.scalar.activation(out=et[:, :, :], in_=s[:, :, :],
                                     func=AF.Exp, scale=SCALE * 0.5)

                def deferred(b=b, hl=hl, q=q, g=g, av=av, et=et, hs=hs, qof=qof):
                    if g < N_FP8_PAIRS:
                        # fp8 DoubleRow m-pair: v split hi/lo across the
                        # slot pair, et fp8 - half the f16 AV cost
                        nc.tensor.matmul(
                            av[:, :], vo8h[b][:, g, :, hs:hs + 128],
                            et[:, :, :], start=(g == 0), stop=False,
                            perf_mode=DR,
                        )
                        nc.tensor.matmul(
                            av[:, :], vo8l[b][:, g, :, hs:hs + 128],
                            et[:, :, :], start=False, stop=(g == NG - 1),
                            perf_mode=DR,
                        )
                    else:
                        for i in range(2):
                            nc.tensor.matmul(
                                av[:, :],
                                vo[b][:, 2 * g + i, hs:hs + 128],
                                et[:, i, :],
                                start=(g == 0 and i == 0),
                                stop=(g == NG - 1 and i == 1),
                            )
                    if g == NG - 1:
                        osl = slice(0, 64) if hl == 0 else slice(64, 128)
                        dsl = slice(64, 128) if hl == 0 else slice(0, 64)
                        rec = rec_pool.tile([128, 512], F32, tag="rec",
                                            name=f"rec_{b}_{hl}_{q}")
                        nc.vector.reciprocal(rec[dsl, :], av[dsl, :])
                        nc.vector.tensor_mul(
                            oc_sb[b][hs:hs + 64, qof:qof + 512],
                            av[osl, :],
                            rec[dsl, :],
                        )
                        if hl == 1:  # batch b's chunk q fully done
                            uq.extend(("proj", u)
                                      for u in emit_proj_units(b, q))

                return deferred

            steps = [(b, hl, q, g)
                     for b in range(B) for hl in range(HPC)
                     for q in range(NCH) for g in range(NG)]
            dq = []          # deferred AV closures, FIFO
            for si, (b, hl, q, g) in enumerate(steps):
                if si == 0:
                    uq.extend(("v0", u) for u in emit_v_units(0))
                if si == 8:
                    # qk chains ahead of v units: scores at si=64 need q/k,
                    # while the v units' consumers (batch-1 AVs) run deep
                    # behind under the large lag, so v work drains into the
                    # ACT-bound batch-1 half
                    uq.extend(("qk1", u) for u in emit_qk_units(1))
                    uq.extend(("v1", u) for u in emit_v_units(1))
                if (b, hl, q, g) == (1, 0, 0, 0):
                    # batch-1 scores need its q/k complete: flush remaining
                    # qk1 units (selectively - v1 may stay queued)
                    rest = []
                    for tag, u in uq:
                        if tag == "qk1":
                            u()
                        else:
                            rest.append((tag, u))
                    uq[:] = rest
                dq.append(attn_scores(b, hl, q, g))
                # inject pending units: fast in the first steps (batch-0
                # qk/v units must beat their consumers), adaptively after
                # 2/step through step 4 so the k(nch2)/k(nch3) evacuations
                # are emitted before the chunk-0 m-sweep reads them (the
                # q(nch*) chains that follow are first read at step 8/16/24)
                npop = 2 if si <= 4 else 1 if si <= 28 else \
                    (1 if (si % 2 == 0 or len(uq) > 20) else 0)
                for _ in range(npop):
                    if uq:
                        uq.pop(0)[1]()
                # run deferred AVs, lagging behind scores/exp so PE never
                # waits on ACT; extra lag early while v-units stream in
                lag = max(2, 30 - max(0, si - 56) // 2)
                while len(dq) > lag:
                    dq.pop(0)()
            while dq:
                dq.pop(0)()
            while uq:
                uq.pop(0)[1]()
    nc.finalize()
    return nc


_NC = None


def _get_nc():
    global _NC
    if _NC is None:
        _NC = _build()
    return _NC


def _make_in_maps(x, w_qkv, w_proj):
    xT = np.ascontiguousarray(x.transpose(0, 2, 1)).astype(np.float16)
    xT = xT.reshape(B, CT, 128, N)
    in_maps = []
    for core in range(NCORES):
        h0 = core * HPC
        rows = np.concatenate(
            [np.arange(h * D, (h + 1) * D) for h in range(h0, h0 + HPC)]
        )
        w = np.concatenate(
            [w_qkv[rows, :], w_qkv[C + rows, :], w_qkv[2 * C + rows, :]], axis=0
        )  # [384, 1024]
        wqkvT = np.ascontiguousarray(w.T).astype(np.float16).reshape(CT, 128, 384)
        cols = np.arange(h0 * D, (h0 + HPC) * D)
        wpT = np.ascontiguousarray(w_proj[:, cols].T).astype(np.float16)
        in_maps.append({"xT": xT, "wqkv": wqkvT, "wpT": wpT})
    return in_maps


def kernel(x, w_qkv, w_proj, b_proj):
    x = np.asarray(x, dtype=np.float32)
    w_qkv = np.asarray(w_qkv, dtype=np.float32)
    w_proj = np.asarray(w_proj, dtype=np.float32)
    b_proj = np.asarray(b_proj, dtype=np.float32)

    in_maps = _make_in_maps(x, w_qkv, w_proj)
    nc = _get_nc()
    res = run_bass_kernel_spmd(nc, in_maps, core_ids=list(range(NCORES)))
    out = np.zeros((B, N, C), dtype=np.float32)
    for core in range(NCORES):
        out += res.results[core]["y"].astype(np.float32)
    out += b_proj
    return out



# revision 32
# speedup vs baseline: 1.0448x; 1.0448x over previous
"""Multi-head attention (B=2, N=2048, C=1024, H=16, D=64) on 8 TRN2 NeuronCores.

Sharding: 2 heads per core (tensor parallel over num_heads), both batch
elements on every core.  Each core computes q/k/v for its 2 heads, full
attention for those heads, and a partial output projection (row-parallel
over w_proj); the host sums the 8 f16 partial outputs and adds the bias.

Device-side dataflow per core:
  q/k:   f16 matmuls over 8 c-tiles into [128,512] PSUM blocks (2 heads x
         64 d on partitions).  q is evacuated to fp8e4m3 duplicated into
         both DoubleRow slots; k is evacuated as a compensated fp8 (hi,
         k-hi_fp8(k)) slot pair, so the score matmuls run in fp8 DoubleRow
         mode (0.5 PE cycles/row, half the f16 cost) with k exact and only
         q carrying ~3.6%-rms quantization noise.  Measured end-to-end
         max-rel-error 1.52e-2 against the f32 reference (budget 2e-2);
         with SCORES_FP8=False everything is f16 at 3.7e-4 error.
  v:     computed directly in [m, d] orientation (x tile as the stationary
         operand) one m-tile at a time, packed into vo tiles laid out
         [V_h0 | ones | V_h1] so each head's AV stationary operand is a
         [128,128] f16 slab whose ones columns produce the softmax
         denominator rows inside the same AV matmul (no PE transposes).
  attn:  one software-pipelined loop over all 128 (batch, head, n-chunk,
         m-pair) steps: two DoubleRow score matmuls (K=64) into a
         [128,2,512] PSUM tile, one ACT exp (scale folded in, f16 out;
         logits are O(3) so no max subtraction is needed), with the AV
         accumulation deferred up to 28 steps behind so PE never waits on
         ACT - across all chunk/head/batch boundaries.
  norm:  DVE reciprocal + multiply into oc (f16).
  proj:  y_partial[n,:] = oc.T @ w_proj in f16, f32 PSUM evacuated to f16,
         DMA'd out per n-tile.

Scheduling: only the first n-chunk's q/k chains of batch 0 run before the
attention step loop.  Everything else (remaining batch-0 q/k chains, both
batches' v units, batch-1 q/k, both projections) is chopped into
~0.4-0.9us PE work units and injected into the attention steps at an
adaptive rate, so the ACT engine's ~134us of exp runs underneath PE's
~145us of matmul work with minimal warmup and drain.  x DMAs are issued
n-half-major (and the qkv weight DMA is split q/k-first) so the first
chains start ~4us into the run; the deep AV lag lets PE batch-0 work run
far ahead of ACT while batch-1's AV stream drains the backlog.
GPSIMD cannot touch PSUM on this hardware, so all PSUM evacuation is on
DVE (plus ACT for the tail n-tiles, where exp is already finished).

Perf notes (timeline-sim traces via the patched LazyPerfetto shim; engine
budgets: PE 138.5us busy, ACT 136.6us busy, total 161.6us).  The run is
ACT-critical end-to-end: first exp t=14.5us, gapless exp 24->151us, then
9.2us drain.  Levers and measured outcomes:
  - Warmup (14.5us to first exp): paced by HWDGE DMA issue (625ns/DMA,
    serialized) - x tiles land one per ~650ns and the first chains run one
    matmul per arrival at 0.65-1.2GHz (p-state resets on every DMA wait).
    Batching x into per-4-ct DMAs + a PE warm-up stream compressed the
    drain to 3.9us but made the early exp stream ragged: net 163.8us,
    REVERTED.  A finer-grained retune of the batched-DMA schedule is the
    most promising open direction.
  - PE p-state: 3us continuous execution required for 2.4GHz; any idle
    resets to 1.2GHz (427 vs 213ns per 512-col matmul, trace-confirmed).
    All PE-work reductions (flipped AV 182us, fp8-DR AV 164.5us) lose by
    starving PE below ACT pace.
  - ACT floor: exp PSUM-capped at [128,2x512] tiles (all 8 banks used);
    mid-run ACT is gapless so only start/drain/busy can improve.
  - kernel_v3_avgquant.py (162375ns): averaged fp8 pairs a=fp8(x),
    b=fp8(2x-a) for both q and k halve the error (HW-verified 7.76e-3 vs
    1.52e-2) at identical PE cost - fallback if the budget tightens.
"""

import sys

sys.path.insert(0, "/opt/trn_rl_repo")

import numpy as np

import concourse.mybir as mybir
import concourse.tile as tile
from concourse import bacc
from concourse.bass_utils import run_bass_kernel_spmd

F32 = mybir.dt.float32
F16 = mybir.dt.float16
F8 = mybir.dt.float8e4
AF = mybir.ActivationFunctionType
ALU = mybir.AluOpType
DR = mybir.MatmulPerfMode.DoubleRow

# fp8 DoubleRow score matmuls: q quantized to fp8e4m3 (both slots), k split
# hi/lo compensated across the two slots -> k exact, q ~3.6% quantization
# noise; halves PE score time.  Validated end-to-end error ~1.5e-2 < 2e-2.
SCORES_FP8 = True

DWARM = 40

B = 2
N = 2048
C = 1024
H = 16
D = 64
NCORES = 8
HPC = H // NCORES          # heads per core = 2
CT = C // 128              # c tiles = 8
NT = N // 128              # m tiles = 16
NP = NT // 2               # m pairs = 8
NCH = N // 512             # 512-wide n chunks = 4
SCALE = float(D) ** -0.5


def _build():
    nc = bacc.Bacc("TRN2")
    xT = nc.dram_tensor("xT", [B, 128, CT, N], F16, kind="ExternalInput")
    wqkv = nc.dram_tensor("wqkv", [128, CT, 384], F16, kind="ExternalInput")
    wpT = nc.dram_tensor("wpT", [128, C], F16, kind="ExternalInput")
    y = nc.dram_tensor("y", [B, N, C], F16, kind="ExternalOutput")

    with tile.TileContext(nc) as tc:
        with tc.tile_pool(name="consts", bufs=1) as consts, \
             tc.tile_pool(name="xt", bufs=2) as xt_pool, \
             tc.tile_pool(name="qk", bufs=4) as qk_pool, \
             tc.tile_pool(name="kf", bufs=2) as kf_pool, \
             tc.tile_pool(name="vo", bufs=2) as vo_pool, \
             tc.tile_pool(name="et", bufs=32) as et_pool, \
             tc.tile_pool(name="oc", bufs=2) as oc_pool, \
             tc.tile_pool(name="rec", bufs=4) as rec_pool, \
             tc.tile_pool(name="yo", bufs=6) as yo_pool, \
             tc.tile_pool(name="pq", bufs=2, space="PSUM") as pq, \
             tc.tile_pool(name="ps", bufs=2, space="PSUM") as ps_pool, \
             tc.tile_pool(name="pav", bufs=2, space="PSUM") as pav:

            w_sb = consts.tile([128, CT, 384], F16)
            wp_sb = consts.tile([128, C], F16)

            xt = {}
            q16 = {}
            k16 = {}
            k16f = {}
            vo = {}
            oc_sb = {}

            def load_w(part):
                # qk columns first (phase-0 critical path); each half
                # interleaved with its x piece since DMA transfers
                # serialize on one lane in the model
                if part == 0:
                    for ts_ in ((0, 4), (4, 8)):
                        nc.sync.dma_start(
                            out=w_sb[:, ts_[0]:ts_[1], 0:256],
                            in_=wqkv[:, ts_[0]:ts_[1], 0:256])
                elif part == 10 or part == 14:
                    c0 = 0 if part == 10 else 4
                    nc.sync.dma_start(
                        out=w_sb[:, c0:c0 + 4, 0:256],
                        in_=wqkv[:, c0:c0 + 4, 0:256])
                else:
                    nc.sync.dma_start(
                        out=w_sb[:, :, 256:384],
                        in_=wqkv[:, :, 256:384])

            def load_x(b, pieces):
                # batched: one DMA per (c-tile group x n-piece); dodges the
                # 625ns/DMA serialized HWDGE issue.
                if b not in xt:
                    xt[b] = xt_pool.tile([128, CT, N], F16, tag="xt",
                                         name=f"xt_{b}")
                for lo, hi, c0, c1 in pieces:
                    nc.sync.dma_start(out=xt[b][:, c0:c1, lo:hi],
                                      in_=xT[b][:, c0:c1, lo:hi])

            def emit_qk_units(b):
                """16 PE units: half-chains of 4 c-tiles for the q and k
                blocks, n-chunk-major so early chains need only early DMAs."""
                if SCORES_FP8:
                    # slot-dim layouts: q duplicated, k as (hi, lo) pair
                    q16[b] = qk_pool.tile([128, 2, N], F8, tag="qk",
                                          name=f"q16_{b}")
                    k16[b] = qk_pool.tile([128, 2, N], F8, tag="qk",
                                          name=f"k16_{b}")
                    k16f[b] = kf_pool.tile([128, N], F16, tag="kf",
                                           name=f"k16f_{b}")
                else:
                    q16[b] = qk_pool.tile([128, N], F16, tag="qk",
                                          name=f"q16_{b}")
                    k16[b] = qk_pool.tile([128, N], F16, tag="qk",
                                          name=f"k16_{b}")

                # q0/k0 first (run before the step loop); then all k chunks
                # ahead of all q chunks: the chunk-0 score m-sweep reads
                # k(nch2) at step 4 and k(nch3) at step 6, while q(nch1) is
                # first read at step 8 - emission must precede those reads
                order = [(0, 0), (1, 0), (1, 1), (1, 2), (1, 3),
                         (0, 1), (0, 2), (0, 3)]
                if True:  # keep indentation stable
                    for blk, nch in order:
                        sl = slice(nch * 512, (nch + 1) * 512)
                        psq = pq.tile([128, 512], F32, tag="pq",
                                      name=f"psq_{b}_{blk}_{nch}")

                        def half(ct0, psq=psq, blk=blk, sl=sl, b=b):
                            for ct in range(ct0, ct0 + 4):
                                nc.tensor.matmul(
                                    psq[:, :],
                                    w_sb[:, ct, blk * 128:(blk + 1) * 128],
                                    xt[b][:, ct, sl],
                                    start=(ct == 0), stop=(ct == CT - 1),
                                )
                            if ct0 != 4:
                                return
                            dst = q16[b] if blk == 0 else k16[b]
                            if not SCORES_FP8:
                                nc.vector.tensor_copy(dst[:, sl], psq[:, :])
                            elif blk == 0:
                                nc.vector.tensor_copy(dst[:, 0, sl], psq[:, :])
                                # slot duplicate from SBUF on the idle Pool
                                # engine (GPSIMD cannot read PSUM) - frees
                                # the PSUM chain slot one copy earlier
                                nc.gpsimd.tensor_copy(dst[:, 1, sl],
                                                      dst[:, 0, sl])
                            else:
                                # single DVE evac to f16, hi/lo split on the
                                # idle Pool engine (SBUF-only) - frees the
                                # PSUM chain slot after one op
                                kf = k16f[b]
                                nc.vector.tensor_copy(kf[:, sl], psq[:, :])
                                nc.gpsimd.tensor_copy(dst[:, 0, sl], kf[:, sl])
                                nc.gpsimd.tensor_tensor(
                                    out=dst[:, 1, sl], in0=kf[:, sl],
                                    in1=dst[:, 0, sl], op=ALU.subtract)

                        yield lambda h=half: h(0)
                        yield lambda h=half: h(4)

            def emit_v_units(b):
                """17 PE units: vo init, then one unit per m-tile computing
                v[m,d] directly (x as stationary operand, 8 accumulating
                128-free matmuls) and packing it into the vo layout."""
                vo[b] = vo_pool.tile([128, NT, 192], F16, tag="vo", name=f"vo_{b}")

                def vo_init(b=b):
                    nc.gpsimd.memset(vo[b][:, :, 64:128], 1.0)

                yield vo_init

                for mt in range(NT):
                    def vunit(mt=mt, b=b):
                        pv = pq.tile([128, 128], F32, tag="pq",
                                     name=f"pv_{b}_{mt}")
                        msl = slice(mt * 128, (mt + 1) * 128)
                        for ct in range(CT):
                            nc.tensor.matmul(
                                pv[:, :],
                                xt[b][:, ct, msl],
                                w_sb[:, ct, 256:384],
                                start=(ct == 0), stop=(ct == CT - 1),
                            )
                        nc.vector.tensor_copy(vo[b][:, mt, 0:64], pv[:, 0:64])
                        nc.vector.tensor_copy(vo[b][:, mt, 128:192], pv[:, 64:128])

                    yield vunit

            def emit_proj_units(b, q):
                """4 PE units: one per n-tile (2 matmuls + evac + DMA out)."""
                for nt in range(q * NT // NCH, (q + 1) * NT // NCH):
                    def unit(nt=nt, b=b, q=q):
                        ysb = yo_pool.tile([128, 1024], F16, tag="yo",
                                           name=f"ysb_{b}_{nt}")
                        for och in range(2):
                            pp = pq.tile([128, 512], F32, tag="pq",
                                         name=f"pp_{b}_{nt}_{och}")
                            nc.tensor.matmul(
                                pp[:, :],
                                oc_sb[b][:, nt * 128:(nt + 1) * 128],
                                wp_sb[:, och * 512:(och + 1) * 512],
                                start=True, stop=True,
                            )
                            cp = (nc.scalar.copy
                                  if (b == 1 and q == 3 and och == nt % 2)
                                  else nc.vector.tensor_copy)
                            cp(ysb[:, och * 512:(och + 1) * 512], pp[:, :])
                        nc.sync.dma_start(
                            out=y[b, nt * 128:(nt + 1) * 128, :],
                            in_=ysb[:, :],
                        )

                    yield unit

            # ---- schedule ----
            if DWARM:
                dwt = consts.tile([128, 128], F16)
                nc.gpsimd.memset(dwt[:, :], 0.0)
                for i_ in range(DWARM):
                    pd = pq.tile([128, 128], F32, tag="pq", name=f"dw_{i_}")
                    nc.tensor.matmul(pd[:, :], dwt[:, :], dwt[:, :],
                                     start=True, stop=True)
            load_w(10)
            load_x(0, [(0, 512, 0, 4)])
            load_w(14)
            load_x(0, [(0, 512, 4, 8)])
            load_x(0, [(512, 1024, 0, 4), (512, 1024, 4, 8),
                       (1024, 1536, 0, 4), (1024, 1536, 4, 8),
                       (1536, 2048, 0, 4), (1536, 2048, 4, 8)])
            load_w(1)
            load_x(1, [(0, 1024, 0, 8), (1024, 2048, 0, 8)])
            nc.sync.dma_start(out=wp_sb, in_=wpT[:, :])

            qk0 = list(emit_qk_units(0))
            for ui in (0, 2, 1, 3):   # q0a, k0a first: they need only ct0-3
                qk0[ui]()

            uq = []          # pending PE work units
            uq.extend(("qk0", u) for u in qk0[4:])
            av_tile = {}

            NG = NP          # m-pair groups per chunk

            def attn_scores(b, hl, q, g):
                """Emit scores for one m-pair + exp; return deferred AV."""
                hs = hl * 64
                qof = q * 512
                if hl == 0 and q == 0 and g == 0:
                    oc_sb[b] = oc_pool.tile([128, N], F16, tag="oc",
                                            name=f"oc_{b}")
                if g == 0:
                    av_tile[b, hl, q] = pav.tile([128, 512], F32, tag="pav",
                                                 name=f"av_{b}_{hl}_{q}")
                av = av_tile[b, hl, q]
                s = ps_pool.tile([128, 2, 512], F32, tag="ps",
                                 name=f"s_{b}_{hl}_{q}_{g}")
                for i in range(2):
                    m_ = 2 * g + i
                    if SCORES_FP8:
                        nc.tensor.matmul(
                            s[:, i, :],
                            k16[b][hs:hs + 64, :, m_ * 128:(m_ + 1) * 128],
                            q16[b][hs:hs + 64, :, qof:qof + 512],
                            start=True, stop=True, perf_mode=DR,
                        )
                    else:
                        nc.tensor.matmul(
                            s[:, i, :],
                            k16[b][hs:hs + 64, m_ * 128:(m_ + 1) * 128],
                            q16[b][hs:hs + 64, qof:qof + 512],
                            start=True, stop=True,
                        )
                et = et_pool.tile([128, 2, 512], F16, tag="et",
                                  name=f"et_{b}_{hl}_{q}_{g}")
                nc.scalar.activation(out=et[:, :, :], in_=s[:, :, :],
                                     func=AF.Exp, scale=SCALE)

                def deferred(b=b, hl=hl, q=q, g=g, av=av, et=et, hs=hs, qof=qof):
                    for i in range(2):
                        nc.tensor.matmul(
                            av[:, :],
                            vo[b][:, 2 * g + i, hs:hs + 128],
                            et[:, i, :],
                            start=(g == 0 and i == 0),
                            stop=(g == NG - 1 and i == 1),
                        )
                    if g == NG - 1:
                        osl = slice(0, 64) if hl == 0 else slice(64, 128)
                        dsl = slice(64, 128) if hl == 0 else slice(0, 64)
                        rec = rec_pool.tile([128, 512], F32, tag="rec",
                                            name=f"rec_{b}_{hl}_{q}")
                        nc.vector.reciprocal(rec[dsl, :], av[dsl, :])
                        nc.vector.tensor_mul(
                            oc_sb[b][hs:hs + 64, qof:qof + 512],
                            av[osl, :],
                            rec[dsl, :],
                        )
                        if hl == 1:  # batch b's chunk q fully done
                            uq.extend(("proj", u)
                                      for u in emit_proj_units(b, q))

                return deferred

            steps = [(b, hl, q, g)
                     for b in range(B) for hl in range(HPC)
                     for q in range(NCH) for g in range(NG)]
            dq = []          # deferred AV closures, FIFO
            for si, (b, hl, q, g) in enumerate(steps):
                if si == 0:
                    uq.extend(("v0", u) for u in emit_v_units(0))
                if si == 8:
                    # qk chains ahead of v units: scores at si=64 need q/k,
                    # while the v units' consumers (batch-1 AVs) run deep
                    # behind under the large lag, so v work drains into the
                    # ACT-bound batch-1 half
                    uq.extend(("qk1", u) for u in emit_qk_units(1))
                    uq.extend(("v1", u) for u in emit_v_units(1))
                if (b, hl, q, g) == (1, 0, 0, 0):
                    # batch-1 scores need its q/k complete: flush remaining
                    # qk1 units (selectively - v1 may stay queued)
                    rest = []
                    for tag, u in uq:
                        if tag == "qk1":
                            u()
                        else:
                            rest.append((tag, u))
                    uq[:] = rest
                dq.append(attn_scores(b, hl, q, g))
                # inject pending units: fast in the first steps (batch-0
                # qk/v units must beat their consumers), adaptively after
                # 2/step through step 4 so the k(nch2)/k(nch3) evacuations
                # are emitted before the chunk-0 m-sweep reads them (the
                # q(nch*) chains that follow are first read at step 8/16/24)
                npop = 2 if si <= 4 else 1 if si <= 28 else \
                    (1 if (si % 2 == 0 or len(uq) > 20) else 0)
                for _ in range(npop):
                    if uq:
                        uq.pop(0)[1]()
                # run deferred AVs, lagging behind scores/exp so PE never
                # waits on ACT; extra lag early while v-units stream in,
                # tail lag 4 keeps the drain slightly denser (-75ns)
                lag = max(4, 30 - max(0, si - 48) // 2)
                while len(dq) > lag:
                    dq.pop(0)()
            while dq:
                dq.pop(0)()
            while uq:
                uq.pop(0)[1]()
    nc.finalize()
    return nc


_NC = None


def _get_nc():
    global _NC
    if _NC is None:
        _NC = _build()
    return _NC


def _make_in_maps(x, w_qkv, w_proj):
    xT = np.ascontiguousarray(x.transpose(0, 2, 1)).astype(np.float16)
    xT = np.ascontiguousarray(xT.reshape(B, CT, 128, N).transpose(0, 2, 1, 3))
    in_maps = []
    for core in range(NCORES):
        h0 = core * HPC
        rows = np.concatenate(
            [np.arange(h * D, (h + 1) * D) for h in range(h0, h0 + HPC)]
        )
        w = np.concatenate(
            [w_qkv[rows, :], w_qkv[C + rows, :], w_qkv[2 * C + rows, :]], axis=0
        )  # [384, 1024]
        wqkvT = np.ascontiguousarray(
            w.T.reshape(CT, 128, 384).transpose(1, 0, 2)).astype(np.float16)
        cols = np.arange(h0 * D, (h0 + HPC) * D)
        wpT = np.ascontiguousarray(w_proj[:, cols].T).astype(np.float16)
        in_maps.append({"xT": xT, "wqkv": wqkvT, "wpT": wpT})
    return in_maps


def kernel(x, w_qkv, w_proj, b_proj):
    x = np.asarray(x, dtype=np.float32)
    w_qkv = np.asarray(w_qkv, dtype=np.float32)
    w_proj = np.asarray(w_proj, dtype=np.float32)
    b_proj = np.asarray(b_proj, dtype=np.float32)

    in_maps = _make_in_maps(x, w_qkv, w_proj)
    nc = _get_nc()
    res = run_bass_kernel_spmd(nc, in_maps, core_ids=list(range(NCORES)))
    out = np.zeros((B, N, C), dtype=np.float32)
    for core in range(NCORES):
        out += res.results[core]["y"].astype(np.float32)
    out += b_proj
    return out

